# revision 24
# baseline (speedup 1.0000x reference)
"""Causal MHSA Trainium2 kernel (8 NeuronCores) — v2.

Sharding: core c = 4*b + g handles batch b and head-group g (4 of 16
heads); host sums the 4 head-group partial projections per batch.

v2 structure (vs v1): one shared PSUM pool (tags qk/s/ctx, 8 banks
total) so phases overlap; emission interleaves qkv-projection blocks
with the attention q-blocks that consume them, and the output
projection with pair-1 attention; causal-diagonal tiles narrow the
exp/matmul widths instead of memsetting; softmax normalization stages
the unnormalized ctx through SBUF so the PSUM slot frees early.
"""

import json

import numpy as np

import concourse.bass as bass
import concourse.mybir as mybir
import concourse.tile as tile
from concourse.bass_utils import run_bass_kernel_spmd

# ---------------------------------------------------------------------------
# Workaround: this container's walrus rejects instructions carrying more
# than one semaphore wait ("Too many sync wait commands", e.g. on the
# TileContext final drain). Split every multi-wait instruction into
# single-wait NoOps on the same engine placed immediately before it.
# ---------------------------------------------------------------------------


def _split_multiwait_bir(bir_bytes: bytes) -> bytes:
    bir = json.loads(bir_bytes)
    ctr = 0
    for fn in bir.get("functions", []):
        for bb in fn.get("blocks", []):
            out = []
            for inst in bb.get("instructions", []):
                si = inst.get("sync_info")
                waits = (si or {}).get("on_wait") or []
                if len(waits) > 1 and "engine" in inst:
                    for w in waits:
                        ctr += 1
                        out.append(
                            {
                                "debug": inst.get("debug", 0),
                                "engine": inst["engine"],
                                "ins": [],
                                "outs": [],
                                "name": f"{inst['name']}-sw{ctr}",
                                "opcode": "NoOp",
                                "sync_info": {"on_update": [], "on_wait": [w]},
                            }
                        )
                    si["on_wait"] = []
                out.append(inst)
            bb["instructions"] = out
    return json.dumps(bir).encode()


class _BassSplitWaits(bass.Bass):
    def to_json_bytes(self) -> bytes:
        return _split_multiwait_bir(super().to_json_bytes())


# ---------------------------------------------------------------------------
B = 2
S = 2048
D = 1024
HD = 64
N_CORES = 8
NHL = 4  # heads per core
E = NHL * HD  # 256
DT = D // 128  # 8
ST = S // 128  # 16
QBS = 512
NQB = S // QBS  # 4
F32 = mybir.dt.float32
F32R = mybir.dt.float32r
SCALE = 1.0 / np.sqrt(HD)


def build_nc() -> bass.Bass:
    nc = _BassSplitWaits()

    x_t = nc.dram_tensor("x_t", [D, S], F32R, kind="ExternalInput")
    wq_t = nc.dram_tensor("wq_t", [D, E], F32R, kind="ExternalInput")
    wk_t = nc.dram_tensor("wk_t", [D, E], F32R, kind="ExternalInput")
    wv_t = nc.dram_tensor("wv_t", [D, E], F32R, kind="ExternalInput")
    wo_t = nc.dram_tensor("wo_t", [E, D], F32R, kind="ExternalInput")
    tri_in = nc.dram_tensor("tri", [128, 128], F32R, kind="ExternalInput")
    ones_in = nc.dram_tensor("ones4", [128, NHL], F32R, kind="ExternalInput")
    out = nc.dram_tensor("out", [S, D], F32, kind="ExternalOutput")
    # DRAM bounce for softmax reciprocal rows: written [1, QBS], read back
    # partition-broadcast to [HD, QBS].
    rbounce = nc.dram_tensor("rbounce", [2, NQB, 2, QBS], F32)

    with tile.TileContext(nc) as tc:
        with (
            tc.tile_pool(name="persist", bufs=1) as pp,
            tc.tile_pool(name="work", bufs=3) as wp,
            tc.tile_pool(name="ps", bufs=1, space="PSUM") as ps,
        ):
            # ---- loads, ordered by first use ----
            def load_w(wdram, nm, eng):
                ts_ = []
                for kt in range(DT):
                    t = pp.tile([128, E], F32R, name=f"{nm}{kt}", tag=f"{nm}{kt}")
                    eng.dma_start(out=t, in_=wdram[kt * 128 : (kt + 1) * 128, :])
                    ts_.append(t)
                return ts_

            xt = [pp.tile([128, S], F32R, name=f"xt{kt}", tag=f"xt{kt}") for kt in range(DT)]

            def load_x(nb, eng=None):
                for kt in range(DT):
                    (eng or nc.sync).dma_start(
                        out=xt[kt][:, nb * QBS : (nb + 1) * QBS],
                        in_=x_t[kt * 128 : (kt + 1) * 128, nb * QBS : (nb + 1) * QBS],
                    )

            # interleave the first-needed tiles (wq[kt], wk[kt], x[kt] chunk
            # of the first q-block) so the first accumulation chain starts
            # as soon as its kt=0 operands land
            wq = [pp.tile([128, E], F32R, name=f"wq{kt}", tag=f"wq{kt}") for kt in range(DT)]
            wk = [pp.tile([128, E], F32R, name=f"wk{kt}", tag=f"wk{kt}") for kt in range(DT)]
            for kt in range(DT):
                nc.sync.dma_start(out=wq[kt], in_=wq_t[kt * 128 : (kt + 1) * 128, :])
                nc.sync.dma_start(out=wk[kt], in_=wk_t[kt * 128 : (kt + 1) * 128, :])
                nc.sync.dma_start(
                    out=xt[kt][:, 0:QBS], in_=x_t[kt * 128 : (kt + 1) * 128, 0:QBS]
                )
            wv = load_w(wv_t, "wv", nc.sync)
            tri = pp.tile([128, 128], F32R, name="tri", tag="tri")
            nc.sync.dma_start(out=tri, in_=tri_in[:, :])
            ones_col = pp.tile([128, NHL], F32R, name="ones_col", tag="ones_col")
            nc.sync.dma_start(out=ones_col, in_=ones_in[:, :])
            load_x(1)
            load_x(2)
            load_x(3)
            wo = []
            for dt_ in range(2):
                t = pp.tile([128, D], F32R, name=f"wo{dt_}", tag=f"wo{dt_}")
                nc.sync.dma_start(out=t, in_=wo_t[dt_ * 128 : (dt_ + 1) * 128, :])
                wo.append(t)

            # ---- persistent intermediates ----
            q_T = [pp.tile([128, S], F32R, name=f"qT{p}", tag=f"qT{p}") for p in range(2)]
            k_T = [pp.tile([128, S], F32R, name=f"kT{p}", tag=f"kT{p}") for p in range(2)]
            v_aug = [
                pp.tile([128, NHL * (HD + 1)], F32R, name=f"va{st}", tag=f"va{st}")
                for st in range(ST)
            ]
            ctx_T = [pp.tile([128, S], F32R, name=f"cT{p}", tag=f"cT{p}") for p in range(2)]

            def qk_proj_one(p: int, nb: int, wlist, dst):
                sl = slice(nb * QBS, (nb + 1) * QBS)
                acc = ps.tile([128, QBS], F32, name="acc", tag="qk", bufs=2)
                for kt in range(DT):
                    nc.tensor.matmul(
                        acc,
                        lhsT=wlist[kt][:, p * 128 : (p + 1) * 128],
                        rhs=xt[kt][:, sl],
                        start=(kt == 0),
                        stop=(kt == DT - 1),
                    )
                nc.vector.tensor_copy(out=dst[p][:, sl], in_=acc)

            def qk_proj(p: int, nb: int):
                qk_proj_one(p, nb, wq, q_T)
                qk_proj_one(p, nb, wk, k_T)

            def v_proj(st: int):
                acc = ps.tile([128, QBS], F32, name="acc", tag="qk", bufs=2)
                psv = acc[:, 0:E]
                for kt in range(DT):
                    nc.tensor.matmul(
                        psv,
                        lhsT=xt[kt][:, st * 128 : (st + 1) * 128],
                        rhs=wv[kt],
                        start=(kt == 0),
                        stop=(kt == DT - 1),
                    )
                # note: memset on an f32r tile fails the walrus ISA check, so
                # the per-head ones column is copied from a host-provided input
                va = v_aug[st].rearrange("p (h c) -> p h c", h=NHL)
                nc.vector.tensor_copy(
                    out=va[:, :, 0:HD], in_=psv.rearrange("p (h c) -> p h c", h=NHL)
                )
                nc.vector.tensor_copy(
                    out=va[:, :, HD : HD + 1],
                    in_=ones_col.rearrange("p (h c) -> p h c", c=1),
                )

            def attention(p: int, qb: int, fillers=()):
                fillers = list(fillers)
                n_kt = 4 * qb + 4
                ctxs = [
                    ps.tile([128, QBS], F32, name=f"ctx{h}", tag="ctx", bufs=2)
                    for h in range(2)
                ]
                pts = {}
                for kt in range(n_kt + 1):
                    if kt < n_kt:
                        o = 0 if kt < 4 * qb else (kt - 4 * qb) * 128
                        w = QBS - o
                        s_ps = ps.tile([128, 2 * QBS], F32, name="s_ps", tag="s", bufs=2)
                        for hl in range(2):
                            nc.tensor.matmul(
                                s_ps[:, hl * QBS + o : (hl + 1) * QBS],
                                lhsT=k_T[p][
                                    hl * HD : (hl + 1) * HD, kt * 128 : (kt + 1) * 128
                                ],
                                rhs=q_T[p][
                                    hl * HD : (hl + 1) * HD,
                                    qb * QBS + o : (qb + 1) * QBS,
                                ],
                                start=True,
                                stop=True,
                            )
                        pt = wp.tile([128, 2 * QBS], F32R, name="pt", tag="pt", bufs=4)
                        sv = s_ps.rearrange("p (h q) -> p h q", h=2)
                        pv = pt.rearrange("p (h q) -> p h q", h=2)
                        nc.scalar.activation(
                            out=pv[:, :, o:QBS],
                            in_=sv[:, :, o:QBS],
                            func=mybir.ActivationFunctionType.Exp,
                            scale=float(SCALE),
                        )
                        if kt >= 4 * qb:
                            for hl in range(2):
                                blk = pt[:, hl * QBS + o : hl * QBS + o + 128]
                                nc.vector.tensor_mul(blk, blk, tri)
                        pts[kt] = (pt, o)
                    if kt > 0:
                        pt, o = pts.pop(kt - 1)
                        for hl in range(2):
                            nc.tensor.matmul(
                                ctxs[hl][0 : HD + 1, o:QBS],
                                lhsT=v_aug[kt - 1][
                                    :, (2 * p + hl) * (HD + 1) : (2 * p + hl + 1) * (HD + 1)
                                ],
                                rhs=pt[:, hl * QBS + o : (hl + 1) * QBS],
                                start=(kt - 1 == 0),
                                stop=(kt - 1 == n_kt - 1),
                                skip_group_check=True,
                            )
                    if fillers:
                        fillers.pop(0)()
                while fillers:
                    fillers.pop(0)()
                # normalize; stage through SBUF so the PSUM slot frees early
                for hl in range(2):
                    cun = wp.tile([HD + 1, QBS], F32, name="cun", tag="cun")
                    nc.vector.tensor_copy(out=cun, in_=ctxs[hl][0 : HD + 1, :])
                    # in-place reciprocal at partition 64 (equal in/out base —
                    # a DVE input at partition 64 with output at partition 0
                    # reads wrong data on HW)
                    nc.vector.reciprocal(
                        out=cun[HD : HD + 1, :], in_=cun[HD : HD + 1, :]
                    )
                    rb = rbounce[p, qb, hl, :]
                    nc.sync.dma_start(out=rb, in_=cun[HD : HD + 1, :])
                    bcast = wp.tile([HD, QBS], F32, name="bcast", tag="bcast")
                    nc.sync.dma_start(
                        out=bcast,
                        in_=bass.AP(
                            tensor=rb.tensor,
                            offset=rb.offset,
                            ap=[[0, HD]] + [list(a) for a in rb.ap],
                        ),
                    )
                    nc.vector.tensor_mul(
                        ctx_T[p][hl * HD : (hl + 1) * HD, qb * QBS : (qb + 1) * QBS],
                        cun[0:HD, :],
                        bcast,
                    )

            def outproj(st: int, split_dma: bool = False):
                osb = wp.tile([128, D], F32, name="osb", tag="osb")
                for nb in range(2):
                    pso = ps.tile([128, QBS], F32, name="pso", tag="qk", bufs=2)
                    for dt_ in range(2):
                        nc.tensor.matmul(
                            pso,
                            lhsT=ctx_T[dt_][:, st * 128 : (st + 1) * 128],
                            rhs=wo[dt_][:, nb * QBS : (nb + 1) * QBS],
                            start=(dt_ == 0),
                            stop=(dt_ == 1),
                        )
                    if nb == 0:
                        nc.vector.tensor_copy(out=osb[:, 0:QBS], in_=pso)
                    else:
                        nc.scalar.copy(out=osb[:, QBS:D], in_=pso)
                    if split_dma:
                        nc.sync.dma_start(
                            out=out[st * 128 : (st + 1) * 128, nb * QBS : (nb + 1) * QBS],
                            in_=osb[:, nb * QBS : (nb + 1) * QBS],
                        )
                if not split_dma:
                    nc.sync.dma_start(out=out[st * 128 : (st + 1) * 128, :], in_=osb)

            # ---- interleaved emission ----
            # Later blocks' projections and the trailing output projection
            # are threaded INSIDE the attention kt-loops, one chunk per kt
            # iteration, so the exp pipeline on ACT never drains while
            # TensorE runs a contiguous block of projection work.
            import functools

            def F(fn, *a):
                return functools.partial(fn, *a)

            qk_proj(0, 0)
            for st in range(4):
                v_proj(st)
            attention(0, 0, [
                F(qk_proj_one, 0, 1, wq, q_T), F(qk_proj_one, 0, 1, wk, k_T),
                F(v_proj, 4), F(v_proj, 5), F(v_proj, 6),
            ])
            attention(0, 1, [
                F(v_proj, 7),
                F(qk_proj_one, 0, 2, wq, q_T), F(qk_proj_one, 0, 2, wk, k_T),
                F(v_proj, 8), F(v_proj, 9), F(v_proj, 10), F(v_proj, 11),
            ])
            attention(0, 2, [
                F(qk_proj_one, 0, 3, wq, q_T), F(qk_proj_one, 0, 3, wk, k_T),
                F(v_proj, 12), F(v_proj, 13), F(v_proj, 14), F(v_proj, 15),
                F(qk_proj_one, 1, 0, wq, q_T), F(qk_proj_one, 1, 0, wk, k_T),
            ])
            attention(0, 3, [
                F(qk_proj_one, 1, 1, wq, q_T), F(qk_proj_one, 1, 1, wk, k_T),
                F(qk_proj_one, 1, 2, wq, q_T), F(qk_proj_one, 1, 2, wk, k_T),
                F(qk_proj_one, 1, 3, wq, q_T), F(qk_proj_one, 1, 3, wk, k_T),
            ])
            # pair-1 blocks run [1, 2, 3, 0] so the kernel ends on the
            # smallest (4-kt) block; each block's outproj group is threaded
            # into the NEXT block's kt slots.
            attention(1, 1)
            attention(1, 2, [F(outproj, st) for st in range(4, 8)])
            attention(1, 3, [F(outproj, st) for st in range(8, 12)])
            attention(1, 0, [F(outproj, st) for st in range(12, 16)])
            for st in range(0, 4):
                outproj(st, split_dma=True)
    return nc


_NC_CACHE = {}


def _get_nc() -> bass.Bass:
    if "nc" not in _NC_CACHE:
        _NC_CACHE["nc"] = build_nc()
    return _NC_CACHE["nc"]


def kernel(in_features: np.ndarray, Wqkv: np.ndarray, Wo: np.ndarray) -> np.ndarray:
    in_features = np.ascontiguousarray(np.asarray(in_features, dtype=np.float32))
    Wqkv = np.asarray(Wqkv, dtype=np.float32)
    Wo = np.asarray(Wo, dtype=np.float32)

    tri = np.triu(np.ones((128, 128), dtype=np.float32))  # P^T[k,q] valid iff q >= k

    in_maps = []
    for c in range(N_CORES):
        b, g = divmod(c, NHL)
        sl = slice(g * E, (g + 1) * E)
        in_maps.append(
            {
                "x_t": np.ascontiguousarray(in_features[b].T),
                "wq_t": np.ascontiguousarray(Wqkv[sl, :].T),
                "wk_t": np.ascontiguousarray(Wqkv[D:][sl, :].T),
                "wv_t": np.ascontiguousarray(Wqkv[2 * D :][sl, :].T),
                "wo_t": np.ascontiguousarray(Wo[:, sl].T),
                "tri": tri,
                "ones4": np.ones((128, NHL), dtype=np.float32),
            }
        )

    res = run_bass_kernel_spmd(_get_nc(), in_maps, core_ids=list(range(N_CORES)))
    outs = [res.results[c]["out"] for c in range(N_CORES)]
    return np.stack(
        [outs[0] + outs[1] + outs[2] + outs[3], outs[4] + outs[5] + outs[6] + outs[7]],
        axis=0,
    )


# revision 26
# speedup vs baseline: 1.0024x; 1.0024x over previous
"""Causal MHSA Trainium2 kernel (8 NeuronCores) — v2.

Sharding: core c = 4*b + g handles batch b and head-group g (4 of 16
heads); host sums the 4 head-group partial projections per batch.

v2 structure (vs v1): one shared PSUM pool (tags qk/s/ctx, 8 banks
total) so phases overlap; emission interleaves qkv-projection blocks
with the attention q-blocks that consume them, and the output
projection with pair-1 attention; causal-diagonal tiles narrow the
exp/matmul widths instead of memsetting; softmax normalization stages
the unnormalized ctx through SBUF so the PSUM slot frees early.
"""

import json

import numpy as np

import concourse.bass as bass
import concourse.mybir as mybir
import concourse.tile as tile
from concourse.bass_utils import run_bass_kernel_spmd

# ---------------------------------------------------------------------------
# Workaround: this container's walrus rejects instructions carrying more
# than one semaphore wait ("Too many sync wait commands", e.g. on the
# TileContext final drain). Split every multi-wait instruction into
# single-wait NoOps on the same engine placed immediately before it.
# ---------------------------------------------------------------------------


def _split_multiwait_bir(bir_bytes: bytes) -> bytes:
    bir = json.loads(bir_bytes)
    ctr = 0
    for fn in bir.get("functions", []):
        for bb in fn.get("blocks", []):
            out = []
            for inst in bb.get("instructions", []):
                si = inst.get("sync_info")
                waits = (si or {}).get("on_wait") or []
                if len(waits) > 1 and "engine" in inst:
                    for w in waits:
                        ctr += 1
                        out.append(
                            {
                                "debug": inst.get("debug", 0),
                                "engine": inst["engine"],
                                "ins": [],
                                "outs": [],
                                "name": f"{inst['name']}-sw{ctr}",
                                "opcode": "NoOp",
                                "sync_info": {"on_update": [], "on_wait": [w]},
                            }
                        )
                    si["on_wait"] = []
                out.append(inst)
            bb["instructions"] = out
    return json.dumps(bir).encode()


class _BassSplitWaits(bass.Bass):
    def to_json_bytes(self) -> bytes:
        return _split_multiwait_bir(super().to_json_bytes())


# ---------------------------------------------------------------------------
B = 2
S = 2048
D = 1024
HD = 64
N_CORES = 8
NHL = 4  # heads per core
E = NHL * HD  # 256
DT = D // 128  # 8
ST = S // 128  # 16
QBS = 512
NQB = S // QBS  # 4
F32 = mybir.dt.float32
F32R = mybir.dt.float32r
SCALE = 1.0 / np.sqrt(HD)


def build_nc() -> bass.Bass:
    nc = _BassSplitWaits()

    x_t = nc.dram_tensor("x_t", [D, S], F32R, kind="ExternalInput")
    wq_t = nc.dram_tensor("wq_t", [D, E], F32R, kind="ExternalInput")
    wk_t = nc.dram_tensor("wk_t", [D, E], F32R, kind="ExternalInput")
    wv_t = nc.dram_tensor("wv_t", [D, E], F32R, kind="ExternalInput")
    wo_t = nc.dram_tensor("wo_t", [E, D], F32R, kind="ExternalInput")
    tri_in = nc.dram_tensor("tri", [128, 128], F32R, kind="ExternalInput")
    ones_in = nc.dram_tensor("ones4", [128, NHL], F32R, kind="ExternalInput")
    out = nc.dram_tensor("out", [S, D], F32, kind="ExternalOutput")
    # DRAM bounce for softmax reciprocal rows: written [1, QBS], read back
    # partition-broadcast to [HD, QBS].
    rbounce = nc.dram_tensor("rbounce", [2, NQB, 2, QBS], F32)

    with tile.TileContext(nc) as tc:
        with (
            tc.tile_pool(name="persist", bufs=1) as pp,
            tc.tile_pool(name="work", bufs=3) as wp,
            tc.tile_pool(name="ps", bufs=1, space="PSUM") as ps,
        ):
            # ---- loads, ordered by first use ----
            def load_w(wdram, nm, eng):
                ts_ = []
                for kt in range(DT):
                    t = pp.tile([128, E], F32R, name=f"{nm}{kt}", tag=f"{nm}{kt}")
                    eng.dma_start(out=t, in_=wdram[kt * 128 : (kt + 1) * 128, :])
                    ts_.append(t)
                return ts_

            xt = [pp.tile([128, S], F32R, name=f"xt{kt}", tag=f"xt{kt}") for kt in range(DT)]

            def load_x(nb, eng=None):
                for kt in range(DT):
                    (eng or nc.sync).dma_start(
                        out=xt[kt][:, nb * QBS : (nb + 1) * QBS],
                        in_=x_t[kt * 128 : (kt + 1) * 128, nb * QBS : (nb + 1) * QBS],
                    )

            # interleave the first-needed tiles (wq[kt], wk[kt], x[kt] chunk
            # of the first q-block) so the first accumulation chain starts
            # as soon as its kt=0 operands land
            wq = [pp.tile([128, E], F32R, name=f"wq{kt}", tag=f"wq{kt}") for kt in range(DT)]
            wk = [pp.tile([128, E], F32R, name=f"wk{kt}", tag=f"wk{kt}") for kt in range(DT)]
            for kt in range(DT):
                nc.sync.dma_start(out=wq[kt], in_=wq_t[kt * 128 : (kt + 1) * 128, :])
                nc.sync.dma_start(out=wk[kt], in_=wk_t[kt * 128 : (kt + 1) * 128, :])
                nc.sync.dma_start(
                    out=xt[kt][:, 0:QBS], in_=x_t[kt * 128 : (kt + 1) * 128, 0:QBS]
                )
            wv = load_w(wv_t, "wv", nc.sync)
            tri = pp.tile([128, 128], F32R, name="tri", tag="tri")
            nc.sync.dma_start(out=tri, in_=tri_in[:, :])
            ones_col = pp.tile([128, NHL], F32R, name="ones_col", tag="ones_col")
            nc.sync.dma_start(out=ones_col, in_=ones_in[:, :])
            load_x(1)
            load_x(2)
            load_x(3)
            wo = []
            for dt_ in range(2):
                t = pp.tile([128, D], F32R, name=f"wo{dt_}", tag=f"wo{dt_}")
                nc.sync.dma_start(out=t, in_=wo_t[dt_ * 128 : (dt_ + 1) * 128, :])
                wo.append(t)

            # ---- persistent intermediates ----
            q_T = [pp.tile([128, S], F32R, name=f"qT{p}", tag=f"qT{p}") for p in range(2)]
            k_T = [pp.tile([128, S], F32R, name=f"kT{p}", tag=f"kT{p}") for p in range(2)]
            v_aug = [
                pp.tile([128, NHL * (HD + 1)], F32R, name=f"va{st}", tag=f"va{st}")
                for st in range(ST)
            ]
            ctx_T = [pp.tile([128, S], F32R, name=f"cT{p}", tag=f"cT{p}") for p in range(2)]

            def qk_proj_one(p: int, nb: int, wlist, dst):
                sl = slice(nb * QBS, (nb + 1) * QBS)
                acc = ps.tile([128, QBS], F32, name="acc", tag="qk", bufs=2)
                for kt in range(DT):
                    nc.tensor.matmul(
                        acc,
                        lhsT=wlist[kt][:, p * 128 : (p + 1) * 128],
                        rhs=xt[kt][:, sl],
                        start=(kt == 0),
                        stop=(kt == DT - 1),
                    )
                nc.vector.tensor_copy(out=dst[p][:, sl], in_=acc)

            def qk_proj(p: int, nb: int):
                qk_proj_one(p, nb, wq, q_T)
                qk_proj_one(p, nb, wk, k_T)

            def v_proj(st: int):
                acc = ps.tile([128, QBS], F32, name="acc", tag="qk", bufs=2)
                psv = acc[:, 0:E]
                for kt in range(DT):
                    nc.tensor.matmul(
                        psv,
                        lhsT=xt[kt][:, st * 128 : (st + 1) * 128],
                        rhs=wv[kt],
                        start=(kt == 0),
                        stop=(kt == DT - 1),
                    )
                # note: memset on an f32r tile fails the walrus ISA check, so
                # the per-head ones column is copied from a host-provided input
                va = v_aug[st].rearrange("p (h c) -> p h c", h=NHL)
                nc.vector.tensor_copy(
                    out=va[:, :, 0:HD], in_=psv.rearrange("p (h c) -> p h c", h=NHL)
                )
                nc.vector.tensor_copy(
                    out=va[:, :, HD : HD + 1],
                    in_=ones_col.rearrange("p (h c) -> p h c", c=1),
                )

            def attention(p: int, qb: int, fillers=()):
                fillers = list(fillers)
                n_kt = 4 * qb + 4
                ctxs = [
                    ps.tile([128, QBS], F32, name=f"ctx{h}", tag="ctx", bufs=2)
                    for h in range(2)
                ]
                pts = {}
                for kt in range(n_kt + 1):
                    if kt < n_kt:
                        o = 0 if kt < 4 * qb else (kt - 4 * qb) * 128
                        w = QBS - o
                        s_ps = ps.tile([128, 2 * QBS], F32, name="s_ps", tag="s", bufs=2)
                        for hl in range(2):
                            nc.tensor.matmul(
                                s_ps[:, hl * QBS + o : (hl + 1) * QBS],
                                lhsT=k_T[p][
                                    hl * HD : (hl + 1) * HD, kt * 128 : (kt + 1) * 128
                                ],
                                rhs=q_T[p][
                                    hl * HD : (hl + 1) * HD,
                                    qb * QBS + o : (qb + 1) * QBS,
                                ],
                                start=True,
                                stop=True,
                            )
                        pt = wp.tile([128, 2 * QBS], F32R, name="pt", tag="pt", bufs=4)
                        sv = s_ps.rearrange("p (h q) -> p h q", h=2)
                        pv = pt.rearrange("p (h q) -> p h q", h=2)
                        nc.scalar.activation(
                            out=pv[:, :, o:QBS],
                            in_=sv[:, :, o:QBS],
                            func=mybir.ActivationFunctionType.Exp,
                            scale=float(SCALE),
                        )
                        if kt >= 4 * qb:
                            for hl in range(2):
                                blk = pt[:, hl * QBS + o : hl * QBS + o + 128]
                                nc.vector.tensor_mul(blk, blk, tri)
                        pts[kt] = (pt, o)
                    if kt > 0:
                        pt, o = pts.pop(kt - 1)
                        for hl in range(2):
                            nc.tensor.matmul(
                                ctxs[hl][0 : HD + 1, o:QBS],
                                lhsT=v_aug[kt - 1][
                                    :, (2 * p + hl) * (HD + 1) : (2 * p + hl + 1) * (HD + 1)
                                ],
                                rhs=pt[:, hl * QBS + o : (hl + 1) * QBS],
                                start=(kt - 1 == 0),
                                stop=(kt - 1 == n_kt - 1),
                                skip_group_check=True,
                            )
                    if fillers:
                        fillers.pop(0)()
                while fillers:
                    fillers.pop(0)()
                # normalize; stage through SBUF so the PSUM slot frees early
                for hl in range(2):
                    cun = wp.tile([HD + 1, QBS], F32, name="cun", tag="cun")
                    nc.vector.tensor_copy(out=cun, in_=ctxs[hl][0 : HD + 1, :])
                    # in-place reciprocal at partition 64 (equal in/out base —
                    # a DVE input at partition 64 with output at partition 0
                    # reads wrong data on HW)
                    nc.vector.reciprocal(
                        out=cun[HD : HD + 1, :], in_=cun[HD : HD + 1, :]
                    )
                    rb = rbounce[p, qb, hl, :]
                    nc.sync.dma_start(out=rb, in_=cun[HD : HD + 1, :])
                    bcast = wp.tile([HD, QBS], F32, name="bcast", tag="bcast")
                    nc.sync.dma_start(
                        out=bcast,
                        in_=bass.AP(
                            tensor=rb.tensor,
                            offset=rb.offset,
                            ap=[[0, HD]] + [list(a) for a in rb.ap],
                        ),
                    )
                    nc.vector.tensor_mul(
                        ctx_T[p][hl * HD : (hl + 1) * HD, qb * QBS : (qb + 1) * QBS],
                        cun[0:HD, :],
                        bcast,
                    )

            def outproj(st: int, split_dma: bool = False):
                osb = wp.tile([128, D], F32, name="osb", tag="osb", bufs=4)
                for nb in range(2):
                    pso = ps.tile([128, QBS], F32, name="pso", tag="qk", bufs=2)
                    for dt_ in range(2):
                        nc.tensor.matmul(
                            pso,
                            lhsT=ctx_T[dt_][:, st * 128 : (st + 1) * 128],
                            rhs=wo[dt_][:, nb * QBS : (nb + 1) * QBS],
                            start=(dt_ == 0),
                            stop=(dt_ == 1),
                        )
                    if nb == 0:
                        nc.vector.tensor_copy(out=osb[:, 0:QBS], in_=pso)
                    else:
                        nc.scalar.copy(out=osb[:, QBS:D], in_=pso)
                    if split_dma:
                        nc.sync.dma_start(
                            out=out[st * 128 : (st + 1) * 128, nb * QBS : (nb + 1) * QBS],
                            in_=osb[:, nb * QBS : (nb + 1) * QBS],
                        )
                if not split_dma:
                    nc.sync.dma_start(out=out[st * 128 : (st + 1) * 128, :], in_=osb)

            # ---- interleaved emission ----
            # Later blocks' projections and the trailing output projection
            # are threaded INSIDE the attention kt-loops, one chunk per kt
            # iteration, so the exp pipeline on ACT never drains while
            # TensorE runs a contiguous block of projection work.
            import functools

            def F(fn, *a):
                return functools.partial(fn, *a)

            qk_proj(0, 0)
            for st in range(4):
                v_proj(st)
            attention(0, 0, [
                F(qk_proj_one, 0, 1, wq, q_T), F(qk_proj_one, 0, 1, wk, k_T),
                F(v_proj, 4), F(v_proj, 5), F(v_proj, 6),
            ])
            attention(0, 1, [
                F(v_proj, 7),
                F(qk_proj_one, 0, 2, wq, q_T), F(qk_proj_one, 0, 2, wk, k_T),
                F(v_proj, 8), F(v_proj, 9), F(v_proj, 10), F(v_proj, 11),
            ])
            attention(0, 2, [
                F(qk_proj_one, 0, 3, wq, q_T), F(qk_proj_one, 0, 3, wk, k_T),
                F(v_proj, 12), F(v_proj, 13), F(v_proj, 14), F(v_proj, 15),
                F(qk_proj_one, 1, 0, wq, q_T), F(qk_proj_one, 1, 0, wk, k_T),
            ])
            attention(0, 3, [
                F(qk_proj_one, 1, 1, wq, q_T), F(qk_proj_one, 1, 1, wk, k_T),
                F(qk_proj_one, 1, 2, wq, q_T), F(qk_proj_one, 1, 2, wk, k_T),
                F(qk_proj_one, 1, 3, wq, q_T), F(qk_proj_one, 1, 3, wk, k_T),
            ])
            # pair-1 blocks run [1, 2, 3, 0] so the kernel ends on the
            # smallest (4-kt) block; each block's outproj group is threaded
            # into the NEXT block's kt slots.
            attention(1, 1)
            attention(1, 2, [F(outproj, st) for st in range(4, 8)])
            attention(1, 3, [F(outproj, st) for st in range(8, 12)])
            attention(1, 0, [F(outproj, st) for st in range(12, 16)])
            for st in range(0, 4):
                outproj(st, split_dma=True)
    return nc


_NC_CACHE = {}


def _get_nc() -> bass.Bass:
    if "nc" not in _NC_CACHE:
        _NC_CACHE["nc"] = build_nc()
    return _NC_CACHE["nc"]


def kernel(in_features: np.ndarray, Wqkv: np.ndarray, Wo: np.ndarray) -> np.ndarray:
    in_features = np.ascontiguousarray(np.asarray(in_features, dtype=np.float32))
    Wqkv = np.asarray(Wqkv, dtype=np.float32)
    Wo = np.asarray(Wo, dtype=np.float32)

    tri = np.triu(np.ones((128, 128), dtype=np.float32))  # P^T[k,q] valid iff q >= k

    in_maps = []
    for c in range(N_CORES):
        b, g = divmod(c, NHL)
        sl = slice(g * E, (g + 1) * E)
        in_maps.append(
            {
                "x_t": np.ascontiguousarray(in_features[b].T),
                "wq_t": np.ascontiguousarray(Wqkv[sl, :].T),
                "wk_t": np.ascontiguousarray(Wqkv[D:][sl, :].T),
                "wv_t": np.ascontiguousarray(Wqkv[2 * D :][sl, :].T),
                "wo_t": np.ascontiguousarray(Wo[:, sl].T),
                "tri": tri,
                "ones4": np.ones((128, NHL), dtype=np.float32),
            }
        )

    res = run_bass_kernel_spmd(_get_nc(), in_maps, core_ids=list(range(N_CORES)))
    outs = [res.results[c]["out"] for c in range(N_CORES)]
    return np.stack(
        [outs[0] + outs[1] + outs[2] + outs[3], outs[4] + outs[5] + outs[6] + outs[7]],
        axis=0,
    )
